# revision 23
# baseline (speedup 1.0000x reference)
"""Trainium2 Bass kernel for nn_LC_Block (gnn_message_passing).

Strategy (pure data-parallel over batch, 2 batches/core on 8 cores):
  - BN1 folded into conv1 weights; temporal conv as Toeplitz matmul on PE
    in bf16 (fp32 matmul is 4 cyc/row, bf16 is 1).
  - ELU computed as elu(x)+1 = max(x+1, min(exp(x), 1)); the +1 is folded
    into the conv bias row and corrected in downstream biases, so only
    ONE fused tensor feeds stage B (half the matmuls of the r/q split).
    Per-block engine recipe alternates ACT-heavy / DVE-heavy to balance.
  - GCN + depthwise-expansion + BN2 scale folded host-side into 16
    accumulating bf16 PE matmuls; BN2 bias applied in the ACT Identity
    drain (accum_out gives the t-sum for channel attention for free).
  - All sigmoids via tanh (sigma(x) = 0.5*tanh(x/2)+0.5, affine folded
    into neighbours) so every ACT func lives in one act table (no
    ACT_TABLE_LOAD on the critical path).
  - Channel max for spatial attention via PE transpose + DVE free-axis
    reduce + transpose back (no slow GPSIMD partition_all_reduce).
  - Consts packed into two DMA loads; big loads spread across engine
    queues so the conv can start ~2us in.
  - Tail (ELU/pool/sep-conv/BN3) on DVE+ACT only, trimmed to the t<896
    region that reaches the output; all pool/BN scales folded host-side.
"""
import numpy as np
import concourse.bass as bass
import concourse.tile as tile
import concourse.mybir as mybir
from concourse.bass_utils import run_bass_kernel_spmd

F32 = mybir.dt.float32
BF16 = mybir.dt.bfloat16
AOP = mybir.AluOpType
AFT = mybir.ActivationFunctionType

B, F1, D, C, T, K = 16, 16, 2, 64, 1000, 64
C2 = F1 * D          # 32
PW = K // 4          # 16
EPS = 1e-5
NCORE = 8
BPC = B // NCORE     # 2
TBLK = 64
NBLK = 16            # covers t 0..1023 (1000 valid)
NF = F1 * TBLK       # 1024 conv out cols per block
H = C2 // 16         # 2 (squeeze-excite hidden per batch)
TS = 897             # t extent needed after channel attention
T5 = 896             # t extent entering the pool/sep tail
NW = 56              # pool blocks needed (t < 896)
NA = 48              # sep-conv output positions needed
ACT_HEAVY = (1, 4, 7, 10, 13, 15)   # blocks whose PSUM drain runs on ACT


def _bf16(a):
    try:
        import ml_dtypes
        return np.asarray(a, np.float32).astype(ml_dtypes.bfloat16)
    except ImportError:
        return np.asarray(a, np.float32)


# ----------------------------------------------------------------- host prep
def _host_consts(inp):
    g1, b1, m1, v1 = (np.asarray(inp[k], np.float32) for k in ('g1', 'b1', 'm1', 'v1'))
    inv1 = g1 / np.sqrt(v1 + EPS)
    w1 = np.asarray(inp['conv1_w'], np.float32)[:, 0, 0, :]
    w1p = w1 * inv1[:, None]
    b1p = (np.asarray(inp['conv1_b'], np.float32) - m1) * inv1 + b1

    Wt = np.zeros((128, NF), np.float32)
    for toff in range(TBLK):
        for ff in range(F1):
            Wt[toff:toff + K, ff * TBLK + toff] = w1p[ff]
    Wt[127, :] = np.repeat(b1p + 1.0, TBLK)       # +1: conv emits cp' = cp+1

    a_hat = np.asarray(inp['a_hat'], np.float32)
    dw_w = np.asarray(inp['dw_w'], np.float32)
    gcn_w = np.asarray(inp['gcn_w'], np.float32)
    gcn_b = np.asarray(inp['gcn_b'], np.float32)
    g2, b2, m2, v2 = (np.asarray(inp[k], np.float32) for k in ('g2', 'b2', 'm2', 'v2'))
    s2 = g2 / np.sqrt(v2 + EPS)
    G = (gcn_w[:, None, None] / F1) * np.einsum('fdc,cj->fdj', dw_w, a_hat)
    Kmat = np.zeros((F1, C, C2), np.float32)          # [f', j, c2]
    for ff in range(F1):
        for d in range(D):
            Kmat[ff, :, ff * D + d] += dw_w[ff, d, :]
    for ff in range(F1):
        for d in range(D):
            Kmat[:, :, ff * D + d] += G[ff, d, :][None, :]
    Kmat *= s2[None, None, :]
    Kstk = np.zeros((128, F1, C2 * BPC), np.float32)  # [(b,j), f', (b,c2)]
    for b in range(BPC):
        Kstk[b * C:(b + 1) * C, :, b * C2:(b + 1) * C2] = np.transpose(Kmat, (1, 0, 2))

    sw = dw_w.sum(-1)
    dw_b = np.asarray(inp['dw_b'], np.float32)
    bias2 = s2 * (dw_b + np.repeat(gcn_b, D) * sw.reshape(-1) - m2) + b2
    corr = Kmat.sum((0, 1))                           # +1-offset through stage B
    b2t = np.tile(bias2 - corr, BPC)

    ca_w1 = np.asarray(inp['ca_w1'], np.float32)
    ca_b1 = np.asarray(inp['ca_b1'], np.float32)
    ca_w2 = np.asarray(inp['ca_w2'], np.float32)
    ca_b2 = np.asarray(inp['ca_b2'], np.float32)
    lca1 = np.zeros((2 * C2, BPC * H), np.float32)    # [(b,c2), (b,h)]
    lca2 = np.zeros((BPC * H, 2 * C2), np.float32)    # [(b,h), (b,c2)]
    for b in range(BPC):
        lca1[b * C2:(b + 1) * C2, b * H:(b + 1) * H] = ca_w1.T
        lca2[b * H:(b + 1) * H, b * C2:(b + 1) * C2] = ca_w2.T

    lmean = np.zeros((64, 2), np.float32)             # [(b,c2), b]
    for b in range(BPC):
        lmean[b * C2:(b + 1) * C2, b] = 1.0

    sa_w = np.asarray(inp['sa_w'], np.float32)
    w6 = sa_w[0, :, 1, :]
    w6adj = w6.copy()
    w6adj[0] /= C2
    lsa = np.zeros((2, 2, 6), np.float32)             # [i][b, (dt,b)]
    for dt in range(3):
        for i in range(2):
            for b in range(BPC):
                lsa[i, b, dt * 2 + b] = w6adj[i, dt]
    sa_g, sa_b, sa_m, sa_v = (float(np.asarray(inp[k]).reshape(-1)[0])
                              for k in ('sa_g', 'sa_b', 'sa_m', 'sa_v'))
    ssa = sa_g / np.sqrt(sa_v + EPS)

    # sigmoid = 0.5*tanh(x/2)+0.5: lbc3 rows 0-1 carry 0.5*broadcast of the
    # tanh; row 2 (driven by a ones row) injects the +0.5.
    lbc3 = np.zeros((3, 64), np.float32)
    for b in range(BPC):
        lbc3[b, b * C2:(b + 1) * C2] = 0.5
    lbc3[2, :] = 0.5

    sep_w = np.asarray(inp['sep_w'], np.float32)[:, 0, 0, :]
    sep_b = np.asarray(inp['sep_b'], np.float32)
    g3, b3, m3, v3 = (np.asarray(inp[k], np.float32) for k in ('g3', 'b3', 'm3', 'v3'))
    s3 = g3 / np.sqrt(v3 + EPS)
    ws = sep_w * s3[:, None] / PW                     # [32, 16]
    bs = s3 * (sep_b - m3) + b3 - PW * ws.sum(1)      # -16*sum: +1-offset corr

    # f32 const pack [64, 22]
    cf = np.zeros((64, 22), np.float32)
    cf[:, 0] = b2t
    cf[0:BPC * H, 1] = np.tile(ca_b1, BPC)
    cf[:, 2] = np.tile(ca_b2, BPC) * 0.5
    cf[0:2, 3] = ssa * 0.5
    cf[0:2, 4] = (sa_b - sa_m * ssa) * 0.5
    cf[:, 5:21] = np.tile(ws, (BPC, 1))
    cf[:, 21] = np.tile(bs, BPC)

    # bf16 const pack [64, 146]
    cb = np.zeros((64, 146), np.float32)
    cb[:, 0:4] = lca1
    cb[0:BPC * H, 4:68] = lca2
    cb[:, 68:70] = lmean
    cb[0:2, 70:76] = lsa[0]
    cb[0:3, 76:140] = lbc3
    cb[0:2, 140:146] = lsa[1]

    wdiag = np.zeros((64, PW * 64), np.float32)       # 16 diag taps for PE
    wst = np.tile(ws, (BPC, 1))
    for k in range(PW):
        wdiag[np.arange(64), 64 * k + np.arange(64)] = wst[:, k]

    return {
        'wtoep': _bf16(Wt),
        'kstk': _bf16(Kstk.reshape(128, F1 * C2 * BPC)),
        'cf': cf,
        'cb': _bf16(cb),
        'id128': _bf16(np.eye(128, dtype=np.float32)),
        'wdiag': _bf16(wdiag),
    }


def _host_xtiles(x, core):
    xc = np.asarray(x, np.float32)[core * BPC:(core + 1) * BPC, 0]  # [2, C, T]
    xTpad = np.zeros((NBLK * TBLK + 128, BPC * C), np.float32)
    xTpad[31:31 + T, :] = xc.reshape(BPC * C, T).T
    tiles = np.zeros((128, NBLK, BPC * C), np.float32)
    for i in range(NBLK):
        tiles[:, i, :] = xTpad[TBLK * i: TBLK * i + 128]
        tiles[127, i, :] = 1.0
    return _bf16(tiles.reshape(128, NBLK * BPC * C))                # [128, 2048]


# ------------------------------------------------------------- device program
_CACHE = {}


def _build_program():
    from concourse import bacc
    nc = bacc.Bacc("TRN2", target_bir_lowering=False, debug=False)
    xt_d = nc.dram_tensor("xt", [128, 2048], BF16, kind="ExternalInput")
    wt_d = nc.dram_tensor("wtoep", [128, NF], BF16, kind="ExternalInput")
    ks_d = nc.dram_tensor("kstk", [128, 1024], BF16, kind="ExternalInput")
    cf_d = nc.dram_tensor("cf", [64, 22], F32, kind="ExternalInput")
    cb_d = nc.dram_tensor("cb", [64, 146], BF16, kind="ExternalInput")
    id_d = nc.dram_tensor("id128", [128, 128], BF16, kind="ExternalInput")
    wd_d = nc.dram_tensor("wdiag", [64, PW * 64], BF16, kind="ExternalInput")
    out_d = nc.dram_tensor("out", [BPC, C2, 3], F32, kind="ExternalOutput")

    with tile.TileContext(nc) as tc:
        with (
            tc.tile_pool(name="sb", bufs=1) as sb,
            tc.tile_pool(name="ep", bufs=3) as ep,
            tc.tile_pool(name="qp", bufs=3) as qp,
            tc.tile_pool(name="yp", bufs=3) as yp,
            tc.tile_pool(name="tmpp", bufs=2) as tmpp,
            tc.tile_pool(name="ps", bufs=2, space="PSUM") as ps,
        ):
            # -------- input loads, spread over engine queues
            wt = sb.tile([128, NF], BF16, tag="wt", name="wt")
            nc.scalar.dma_start(wt[:, 0:512], wt_d.ap()[:, 0:512])
            nc.scalar.dma_start(wt[:, 512:1024], wt_d.ap()[:, 512:1024])
            xsb = sb.tile([128, 2048], BF16, tag="xt", name="xt_sb")
            nc.sync.dma_start(xsb[:, 0:128], xt_d.ap()[:, 0:128])
            nc.sync.dma_start(xsb[:, 128:1024], xt_d.ap()[:, 128:1024])
            nc.sync.dma_start(xsb[:, 1024:2048], xt_d.ap()[:, 1024:2048])
            ks = sb.tile([128, 1024], BF16, tag="ks", name="ks_sb")
            nc.scalar.dma_start(ks[:], ks_d.ap())
            neg1 = sb.tile([128, 1], F32, tag="neg1", name="neg1")
            nc.gpsimd.memset(neg1[:], -1.0)
            cft = sb.tile([64, 22], F32, tag="cf", name="cf_sb")
            nc.gpsimd.dma_start(cft[:], cf_d.ap())
            cbt = sb.tile([64, 146], BF16, tag="cb", name="cb_sb")
            nc.gpsimd.dma_start(cbt[:], cb_d.ap())
            idt = sb.tile([128, 128], BF16, tag="id", name="id_sb")
            nc.gpsimd.dma_start(idt[:], id_d.ap())
            wdt = sb.tile([64, PW * 64], BF16, tag="wd", name="wd_sb")
            nc.gpsimd.dma_start(wdt[:], wd_d.ap())

            b2t = cft[:, 0:1]
            bca1 = cft[0:BPC * H, 1:2]
            bca2h = cft[:, 2:3]
            ssat = cft[0:2, 3:4]
            bsat = cft[0:2, 4:5]
            wsept = cft[:, 5:21]
            bsept = cft[:, 21:22]
            lca1 = cbt[:, 0:4]
            lca2 = cbt[0:BPC * H, 4:68]
            lmt = cbt[:, 68:70]
            lsam = cbt[0:2, 70:76]
            lsax = cbt[0:2, 140:146]
            lbc3 = cbt[0:3, 76:140]

            el = sb.tile([128, NBLK * NF], BF16, tag="el", name="el")
            h4 = sb.tile([64, 1024], BF16, tag="h4", name="h4")
            nc.gpsimd.memset(h4[:], 0.0)
            scmean = sb.tile([2, TS + 2], BF16, tag="scmean", name="scmean")
            nc.gpsimd.memset(scmean[:], 0.0)
            scmax = sb.tile([2, TS + 2], BF16, tag="scmax", name="scmax")
            nc.gpsimd.memset(scmax[:], 0.0)
            tt5 = sb.tile([3, T5], BF16, tag="tt5", name="tt5")
            nc.gpsimd.memset(tt5[:], 1.0)   # rows 0-1 overwritten by tanh
            p_pad = sb.tile([64, 63], BF16, tag="ppad", name="ppad")
            nc.gpsimd.memset(p_pad[:], float(PW))

            # ---- conv1 (+1 in bias row) + ELU' = max(cp', min(exp(cp'-1),1))
            for i in range(NBLK):
                cp = ps.tile([128, NF], F32, tag="cp", name="cp", bufs=3)
                lhs = xsb[:, 128 * i:128 * (i + 1)]
                nc.tensor.matmul(cp[:, 0:512], lhs, wt[:, 0:512])
                nc.tensor.matmul(cp[:, 512:1024], lhs, wt[:, 512:1024])
                e_t = ep.tile([128, NF], BF16, tag="e", name="e")
                nc.scalar.activation(e_t[:], cp[:], AFT.Exp, bias=neg1[:])
                q_t = qp.tile([128, NF], BF16, tag="q", name="q")
                nc.vector.tensor_scalar(q_t[:], e_t[:], 1.0, None, op0=AOP.min)
                dst = el[:, NF * i:NF * (i + 1)]
                if i in ACT_HEAVY:  # drain cp' via ACT, combine all-bf16
                    y_t = yp.tile([128, NF], BF16, tag="y", name="y")
                    nc.scalar.activation(y_t[:], cp[:], AFT.Copy)
                    nc.vector.tensor_tensor(dst, y_t[:], q_t[:], op=AOP.max)
                else:               # combine straight from PSUM on DVE
                    nc.vector.tensor_tensor(dst, cp[:], q_t[:], op=AOP.max)

            # ---- fused GCN + depthwise-expansion matmuls; BN2 bias in drain
            elv = el[:].rearrange("p (blk f toff) -> p f blk toff", blk=NBLK, f=F1)
            h3b = sb.tile([64, T], BF16, tag="h3b", name="h3b")
            casum = [sb.tile([64, 1], F32, tag=f"cas{q}", name=f"cas{q}")
                     for q in range(4)]
            hmaxh = [sb.tile([64, 1], F32, tag=f"hmx{q}", name=f"hmx{q}")
                     for q in range(4)]
            for q in range(4):
                dp = ps.tile([64, 256], F32, tag="t", name="dp")
                for fp in range(F1):
                    nc.tensor.matmul(dp[:], ks[:, 64 * fp:64 * (fp + 1)],
                                     elv[:, fp, 4 * q:4 * (q + 1), :],
                                     start=(fp == 0), stop=(fp == F1 - 1))
                w = 256 if q < 3 else T - 768
                nc.scalar.activation(h3b[:, 256 * q:256 * q + w], dp[:, 0:w],
                                     AFT.Identity, bias=b2t,
                                     accum_out=casum[q][:])
                nc.vector.tensor_reduce(hmaxh[q][:], h3b[:, 256 * q:256 * q + w],
                                        axis=mybir.AxisListType.X, op=AOP.max)

            # ---- channel attention (sigmoid via tanh)
            hm1 = sb.tile([64, 1], F32, tag="hm1", name="hm1")
            nc.vector.tensor_tensor(hm1[:], hmaxh[0][:], hmaxh[1][:], op=AOP.max)
            nc.vector.tensor_tensor(hm1[:], hm1[:], hmaxh[2][:], op=AOP.max)
            hmax = sb.tile([64, 1], F32, tag="hmax", name="hmax")
            nc.vector.tensor_tensor(hmax[:], hm1[:], hmaxh[3][:], op=AOP.max)
            sa1 = sb.tile([64, 1], F32, tag="sa1", name="sa1")
            nc.vector.tensor_tensor(sa1[:], casum[0][:], casum[1][:], op=AOP.add)
            nc.vector.tensor_tensor(sa1[:], sa1[:], casum[2][:], op=AOP.add)
            s1 = sb.tile([64, 1], F32, tag="s1", name="s1")
            nc.vector.tensor_tensor(s1[:], sa1[:], casum[3][:], op=AOP.add)
            s3t = sb.tile([64, 1], BF16, tag="s3t", name="s3t")
            nc.vector.tensor_scalar(s3t[:], s1[:], 1.0 / T, hmax[:],
                                    op0=AOP.mult, op1=AOP.add)
            p1 = ps.tile([4, 1], F32, tag="t", name="p1")
            nc.tensor.matmul(p1[:], lca1[:], s3t[:])
            u = sb.tile([4, 1], BF16, tag="u", name="u")
            nc.vector.tensor_scalar(u[:], p1[:], bca1, 0.0, op0=AOP.add,
                                    op1=AOP.max)
            p2 = ps.tile([64, 1], F32, tag="t", name="p2")
            nc.tensor.matmul(p2[:], lca2[:], u[:])
            attt = sb.tile([64, 1], F32, tag="attt", name="attt")
            nc.scalar.activation(attt[:], p2[:], AFT.Tanh, bias=bca2h,
                                 scale=0.5)
            att = sb.tile([64, 1], F32, tag="att", name="att")
            nc.vector.tensor_scalar(att[:], attt[:], 0.5, 0.5,
                                    op0=AOP.mult, op1=AOP.add)
            nc.vector.tensor_scalar(h4[:, 0:512], h3b[:, 0:512], att[:], None,
                                    op0=AOP.mult)
            nc.vector.tensor_scalar(h4[:, 512:TS], h3b[:, 512:TS], att[:], None,
                                    op0=AOP.mult)

            # ---- spatial attention: mean via PE, channel max via transpose
            for a, b in ((0, 449), (449, TS)):
                sp = ps.tile([2, 512], F32, tag="t", name="sp")
                nc.tensor.matmul(sp[:, 0:b - a], lmt[:], h4[:, a:b])
                nc.scalar.activation(scmean[:, 1 + a:1 + b], sp[:, 0:b - a],
                                     AFT.Copy)
            maxc = sb.tile([128, 16], BF16, tag="maxc", name="maxc")
            for p in range(4):
                trc = ps.tile([128, 128], BF16, tag="t", name="trc")
                for g in range(2):
                    nc.tensor.transpose(trc[:, 64 * g:64 * (g + 1)],
                                        h4[:, 256 * p + 128 * g:
                                            256 * p + 128 * (g + 1)],
                                        idt[0:64, 0:64])
                nc.vector.tensor_reduce(
                    maxc[:].rearrange("p (b k) -> p k b", b=BPC)[:, 2 * p:2 * p + 2, :],
                    trc[:].rearrange("p (g b c) -> p g b c", g=2, b=BPC),
                    axis=mybir.AxisListType.X, op=AOP.max)
            mtr = ps.tile([16, 128], BF16, tag="t", name="mtr")
            nc.tensor.transpose(mtr[:], maxc[:], idt[:])
            mts = sb.tile([16, 128], BF16, tag="mts", name="mts")
            nc.scalar.activation(mts[:], mtr[:], AFT.Copy)
            nc.sync.dma_start(
                scmax[:, 1:1 + TS],
                mts[:].rearrange("(b k) t -> b (k t)", b=BPC)[:, 0:TS])

            # ---- 3-tap sa conv; sigmoid as 0.5*tanh+0.5 folded into bp matmul
            for a, b in ((0, 448), (448, T5)):
                pp = ps.tile([2, 512], F32, tag="t", name="pp")
                n = 0
                for lsat, srct in ((lsam, scmean), (lsax, scmax)):
                    for dt in range(3):
                        nc.tensor.matmul(pp[:, 0:b - a],
                                         lsat[:, 2 * dt:2 * dt + 2],
                                         srct[:, a + dt:b + dt],
                                         start=(n == 0), stop=(n == 5))
                        n += 1
                nc.scalar.activation(tt5[0:2, a:b], pp[:, 0:b - a], AFT.Tanh,
                                     bias=bsat, scale=ssat)

            # ---- h5 = h4 * sigma; ELU'; pool(16)
            h5 = sb.tile([64, T5], BF16, tag="h5", name="h5")
            e5 = ep.tile([64, T5], BF16, tag="e5", name="e5")
            r5 = qp.tile([64, T5], BF16, tag="r5", name="r5")
            q5 = yp.tile([64, T5], BF16, tag="q5", name="q5")
            el5 = sb.tile([64, T5], BF16, tag="el5", name="el5")
            for a, b in ((0, 448), (448, T5)):
                bp = ps.tile([64, 512], F32, tag="t", name="bp")
                nc.tensor.matmul(bp[:, 0:b - a], lbc3[:], tt5[:, a:b])
                nc.vector.tensor_tensor(h5[:, a:b], h4[:, a:b], bp[:, 0:b - a],
                                        op=AOP.mult)
                nc.scalar.activation(e5[:, a:b], h5[:, a:b], AFT.Exp)
                nc.vector.tensor_scalar(r5[:, a:b], h5[:, a:b], 0.0, None,
                                        op0=AOP.max)
                nc.vector.tensor_scalar(q5[:, a:b], e5[:, a:b], 1.0, None,
                                        op0=AOP.min)
                nc.vector.tensor_tensor(el5[:, a:b], r5[:, a:b], q5[:, a:b],
                                        op=AOP.add)
                w0 = 7 + a // PW
                with nc.allow_low_precision(reason="16-wide pool sum in bf16"):
                    nc.vector.tensor_reduce(
                        p_pad[:, w0:w0 + (b - a) // PW],
                        el5[:, a:b].rearrange("p (w k) -> p w k", k=PW),
                        axis=mybir.AxisListType.X, op=AOP.add)

            # ---- separable temporal conv (+BN3) as 16 diagonal PE matmuls
            aps = ps.tile([64, NA], F32, tag="t", name="aps")
            for k in range(PW):
                nc.tensor.matmul(aps[:], wdt[:, 64 * k:64 * (k + 1)],
                                 p_pad[:, k:k + NA],
                                 start=(k == 0), stop=(k == PW - 1))
            acc = sb.tile([64, NA], BF16, tag="sacc", name="sacc")
            nc.scalar.activation(acc[:], aps[:], AFT.Identity, bias=bsept)

            # ---- final ELU' + pool(16), -1 offset folded into the /16 step
            e6 = ep.tile([64, NA], BF16, tag="e6", name="e6")
            nc.scalar.activation(e6[:], acc[:], AFT.Exp)
            r6 = qp.tile([64, NA], BF16, tag="r6", name="r6")
            nc.vector.tensor_scalar(r6[:], acc[:], 0.0, None, op0=AOP.max)
            q6 = yp.tile([64, NA], BF16, tag="q6", name="q6")
            nc.vector.tensor_scalar(q6[:], e6[:], 1.0, None, op0=AOP.min)
            el6 = sb.tile([64, NA], BF16, tag="el6", name="el6")
            nc.vector.tensor_tensor(el6[:], r6[:], q6[:], op=AOP.add)
            po = sb.tile([64, 3], F32, tag="po", name="po")
            nc.vector.tensor_reduce(po[:],
                                    el6[:].rearrange("p (w k) -> p w k", k=PW),
                                    axis=mybir.AxisListType.X, op=AOP.add)
            ot = sb.tile([64, 3], F32, tag="ot", name="ot")
            nc.vector.tensor_scalar(ot[:], po[:], 1.0 / PW, -1.0,
                                    op0=AOP.mult, op1=AOP.add)
            nc.sync.dma_start(out_d.ap().rearrange("b c w -> (b c) w"), ot[:])
    nc.compile()
    return nc


def kernel(**inputs):
    if 'nc' not in _CACHE:
        _CACHE['nc'] = _build_program()
    nc = _CACHE['nc']
    consts = _host_consts(inputs)
    in_maps = []
    for core in range(NCORE):
        m = dict(consts)
        m['xt'] = _host_xtiles(inputs['x'], core)
        in_maps.append(m)
    res = run_bass_kernel_spmd(nc, in_maps, list(range(NCORE)))
    globals()['_LAST_RES'] = res
    out = np.concatenate([np.asarray(res.results[i]['out'])
                          for i in range(NCORE)], axis=0)
    return out.astype(np.float32)


if __name__ == '__main__':
    d = np.load('/root/problem/ref_data.npz')
    inputs = {k: d[k] for k in d.files if k != 'expected'}
    out = kernel(**inputs)
    exp = d['expected']
    err = np.abs(out - exp).max() / (np.abs(exp).max() + 1e-9)
    print('out', out.shape, 'rel(absmax) err', err)
